# revision 18
# baseline (speedup 1.0000x reference)
"""Trainium2 Bass kernel for CoordsTo_atomNRF.

out[b, i] = (1/(4*MAX_ATOM_NRF)) * sum_{pairs p containing i} atoms_flat[p]*AU2KCALMOLA / r2[b, p]

Per core (batch sharded 8 ways, 1024 examples each):
  - DMA coords b-tiles [128 ex, 192] contiguous; PE transposes (strided
    in-AP per xyz plane) -> atom-major planes C [64, 3, 512] in SBUF.
  - Per chunk of 126 pairs: constant E-matmul (lhsT [64,126] of +-1, f32r)
    forms diffs dx/dy/dz in PSUM; squares+sums split across ACT/DVE/GPSIMD
    (alternating per chunk); custom fast reciprocal on DVE; scatter matmul
    with weights A (atoms_flat * AU2K / 400 folded in) accumulates per-atom
    output in PSUM.
  - Output kept atom-major [64, 1024]; host transposes.

Handles two toolchain quirks post-trace: custom-DVE ISA bytes must be
generated explicitly, and every instruction may carry at most one sync wait
(zero for raw ISA) -- surplus waits are split onto same-engine NoOps.
"""

import numpy as np

import concourse.bass as bass
import concourse.mybir as mybir
from concourse.bass_utils import run_bass_kernel_spmd
from concourse.masks import make_identity
from concourse.tile import TileContext

AU2KCALMOLA = 627.5095 * 0.529177
MAX_ATOM_NRF = 100.0
N = 64
NC2 = 2016
CH = 126          # pairs per chunk
NCHUNK = 16
NCORES = 8
B = 8192
BC = B // NCORES  # 1024 examples per core
BG = 512          # examples per matmul group (PSUM bank width in fp32)
NBG = BC // BG    # 2
NBT = BG // 128   # 4 b-tiles per group

F32 = mybir.dt.float32
F32R = mybir.dt.float32r
USE_F32R = True

# tunables for sim sweeps
OPTS = {
    "wpool_bufs": 2,
    "io_bufs": 3,
    "pl_bufs": 2,
    "path": "alt",  # "alt" | "A" | "B"
    "skip_diff": False,
    "skip_elem": False,
    "skip_scatter": False,
    "skip_stage1": False,
}


def _build_weights(atoms_flat):
    i_idx, j_idx = np.tril_indices(N, -1)
    cp = (atoms_flat.astype(np.float64) * AU2KCALMOLA / (4.0 * MAX_ATOM_NRF)).astype(
        np.float32
    )
    p = np.arange(NC2)
    c, m = p // CH, p % CH
    ew = np.zeros((2 * N, NC2), np.float32)
    ew[i_idx, p] = 1.0
    ew[j_idx, p] = -1.0
    ew[N + i_idx, p] = 1.0
    ew[N + j_idx, p] = -1.0
    aw = np.zeros((CH, NCHUNK * N), np.float32)
    np.add.at(aw, (m, c * N + i_idx), cp)
    np.add.at(aw, (m, c * N + j_idx), cp)
    return ew, aw


_SQADD = None


def _get_sqadd():
    """Register a custom DVE op: out = Src0*Src0 + Src1 (sha computed at
    runtime so we don't depend on pinned hashes)."""
    global _SQADD
    if _SQADD is not None:
        return _SQADD
    from concourse import dve_ops
    from concourse.dve_spec import Spec, Src0, Src1, _has_src1, lower
    from concourse.dve_table_gen import dve_ver_for
    from concourse.dve_uop import DveOpSpec

    name = "SQADD_KRN"
    existing = [op for op in dve_ops.OPS if op.name == name]
    if existing:
        _SQADD = existing[0]
        return _SQADD

    spec = Spec(
        body=Src0 * Src0 + Src1,
        reference=lambda in0, in1, s0, s1, imm2: (
            in0.astype(np.float32) * in0.astype(np.float32) + in1
        ).astype(np.float32),
    )
    row = max(dve_ops._SUB_OPCODE_FOR_NAME.values()) + 1
    shas = {}
    for ver in ("v3", "v4"):
        try:
            uops = lower(spec, ver=ver)
            shas[ver] = DveOpSpec(
                name=name, opcode=row, uops=uops, rd1_en=_has_src1(spec)
            ).sha(ver)
        except Exception:
            pass
    op = dve_ops.DveOp(name, spec, subdim=False, uops_sha=shas)
    dve_ops.OPS.append(op)
    dve_ops.CUSTOM_DVE_SPECS[name] = spec
    dve_ops._SUB_OPCODE_FOR_NAME[name] = row
    _SQADD = op
    return op


_NC = None


def _get_nc():
    global _NC
    if _NC is None:
        _NC = _build_nc()
        mybir.codegen_inst_isa_subclasses(_NC)  # fill custom-DVE ISA bytes
        _split_waits(_NC)
    return _NC


def _split_waits(nc):
    """This toolchain allows at most one sync wait per instruction (zero on
    raw ISA instructions). Move surplus waits onto same-engine NoOps."""
    n = 0
    for fn in nc.m.functions:
        for bb in fn.blocks:
            insts = list(bb.instructions)
            out = []
            changed = False
            for inst in insts:
                si = inst.sync_info
                maxw = 0 if inst.opcode == "ISA" else 1
                if si is not None and si.on_wait and len(si.on_wait) > maxw:
                    waits = list(si.on_wait)
                    keep = waits[len(waits) - maxw :] if maxw else []
                    move = waits[: len(waits) - maxw] if maxw else waits
                    for w in move:
                        nop = mybir.InstNoOp(
                            name=f"{inst.name}-pw{n}", ins=[], outs=[]
                        )
                        n += 1
                        nop.engine = inst.engine
                        nop.sync_info = mybir.SyncInfo(on_wait=[w], on_update=[])
                        out.append(nop)
                    si.on_wait = keep
                    changed = True
                out.append(inst)
            if changed:
                bb.instructions = out


def _r(ap):
    return ap.bitcast(F32R) if USE_F32R else ap




def _recip_fast(nc, inv, r2):
    """reciprocal_approx_fast with an f32r-typed output tile (the stock
    wrapper asserts fp32 out; the ISA op itself doesn't care)."""
    from concourse.dve_ops import RECIP_APPROX_FAST_CONSTS, RECIPROCAL_APPROX_FAST

    c = RECIP_APPROX_FAST_CONSTS
    nc.vector._custom_dve(
        RECIPROCAL_APPROX_FAST,
        out=inv[:],
        in0=r2[:],
        s0=c["s0"],
        s1=c["s1"],
        imm2=c["imm2"],
    )

def _build_nc():
    sqadd = _get_sqadd()
    nc = bass.Bass()
    coords_d = nc.declare_dram_parameter("coords", [BC, N * 3], F32, isOutput=False)
    ew_d = nc.declare_dram_parameter("ew", [2 * N, NC2], F32R, isOutput=False)
    aw_d = nc.declare_dram_parameter("aw", [CH, NCHUNK * N], F32R, isOutput=False)
    out_d = nc.declare_dram_parameter("out", [N, BC], F32, isOutput=True)

    Sq = mybir.ActivationFunctionType.Square

    with TileContext(nc) as tc:
        with (
            tc.tile_pool(name="const", bufs=1) as cpool,
            tc.tile_pool(name="io", bufs=OPTS["io_bufs"]) as iopool,
            tc.tile_pool(name="planes", bufs=OPTS["pl_bufs"]) as plpool,
            tc.tile_pool(name="work", bufs=OPTS["wpool_bufs"]) as wpool,
            tc.tile_pool(name="invp", bufs=NCHUNK + 2) as ipool,
            tc.tile_pool(name="pst", bufs=1, space="PSUM") as pst,
            tc.tile_pool(name="psd", bufs=2, space="PSUM") as psd,
            tc.tile_pool(name="pso", bufs=1, space="PSUM") as pso,
        ):
            ident = cpool.tile([128, 128], F32)
            make_identity(nc, ident[:])
            ew_sb = cpool.tile([2 * N, NC2], F32R)
            nc.sync.dma_start(out=ew_sb[:], in_=ew_d[:])
            aw_sb = cpool.tile([CH, NCHUNK * N], F32R)
            nc.sync.dma_start(out=aw_sb[:], in_=aw_d[:])

            for g in range(NBG):
                # stage 1: load + transpose to atom-major planes, splitting
                # fp32 coords into f32r hi (ACT copy rounds) + exact residual
                # lo (DVE sub) so two 1-cyc/row f32r matmuls reproduce full
                # fp32 diffs.
                Cb = plpool.tile([2 * N, 3, BG], F32R)
                for t in range(NBT):
                    raw = iopool.tile([128, N * 3], F32)
                    row0 = (g * NBT + t) * 128
                    nc.sync.dma_start(out=raw[:], in_=coords_d[row0 : row0 + 128, :])
                    rawv = raw[:].rearrange("p (i k) -> p k i", k=3)  # [128,3,64]
                    tp = pst.tile([N, 3 * 128], F32)
                    tpv = tp[:].rearrange("p (k c) -> p k c", k=3)
                    for k in range(3 if not OPTS["skip_stage1"] else 0):
                        nc.tensor.transpose(
                            out=tpv[:, k, :],
                            in_=rawv[:, k, :],
                            identity=ident[:],
                        )
                    if not OPTS["skip_stage1"]:
                        hi = Cb[0:N, :, t * 128 : (t + 1) * 128]
                        nc.scalar.copy(out=hi, in_=tpv[:])
                        nc.vector.tensor_tensor(
                            out=Cb[N : 2 * N, :, t * 128 : (t + 1) * 128],
                            in0=tpv[:],
                            in1=hi.bitcast(F32),
                            op=mybir.AluOpType.subtract,
                        )

                # stage 2: chunk loop (scatters deferred so the in-order
                # PE queue never head-blocks on the elementwise chain)
                outp = pso.tile([N, BG], F32)
                invs = []
                for c in range(NCHUNK):
                    dps = psd.tile([CH, 3 * BG], F32)
                    for k in range(3 if not OPTS["skip_diff"] else 0):
                        nc.tensor.matmul(
                            out=dps[:, k * BG : (k + 1) * BG],
                            lhsT=ew_sb[:, c * CH : (c + 1) * CH],
                            rhs=Cb[:, k, :],
                            start=True,
                            stop=True,
                        )
                    dx = dps[:, 0:BG]
                    dy = dps[:, BG : 2 * BG]
                    dz = dps[:, 2 * BG : 3 * BG]
                    if OPTS["skip_elem"]:
                        continue
                    inv = ipool.tile([CH, BG], F32R)
                    # balanced split: ACT squares x/z, DVE fuses y-square+add
                    # and the reciprocal, GPSIMD does the final combine
                    sqx = wpool.tile([CH, BG], F32)
                    sqz = wpool.tile([CH, BG], F32)
                    u = wpool.tile([CH, BG], F32)
                    r2 = wpool.tile([CH, BG], F32)
                    nc.scalar.activation(sqx[:], dx, Sq)
                    nc.scalar.activation(sqz[:], dz, Sq)
                    nc.vector._custom_dve(sqadd, out=u[:], in0=dy, in1=sqx[:])
                    nc.gpsimd.tensor_add(out=r2[:], in0=u[:], in1=sqz[:])
                    _recip_fast(nc, inv, r2)
                    invs.append(inv)
                for c in range(NCHUNK if not OPTS["skip_scatter"] else 0):
                    nc.tensor.matmul(
                        out=outp[:],
                        lhsT=aw_sb[:, c * N : (c + 1) * N],
                        rhs=invs[c][:],
                        start=(c == 0),
                        stop=(c == NCHUNK - 1),
                    )

                if not OPTS["skip_scatter"]:
                    osb = iopool.tile([N, BG], F32)
                    nc.scalar.copy(out=osb[:], in_=outp[:])
                    nc.sync.dma_start(
                        out=out_d[:, g * BG : (g + 1) * BG], in_=osb[:]
                    )

    return nc


def run(coords, atoms_flat, trace=False):
    coords = np.ascontiguousarray(np.asarray(coords, dtype=np.float32))
    atoms_flat = np.asarray(atoms_flat, dtype=np.float32)
    assert coords.shape == (B, N, 3), coords.shape
    ew, aw = _build_weights(atoms_flat)
    nc = _get_nc()
    in_maps = [
        {
            "coords": coords[c * BC : (c + 1) * BC].reshape(BC, N * 3),
            "ew": ew,
            "aw": aw,
        }
        for c in range(NCORES)
    ]
    res = run_bass_kernel_spmd(nc, in_maps, list(range(NCORES)), trace=trace)
    out = np.empty((B, N), np.float32)
    for c in range(NCORES):
        out[c * BC : (c + 1) * BC] = res.results[c]["out"].T
    return out, res


def kernel(coords, atoms_flat):
    out, _ = run(coords, atoms_flat, trace=False)
    return out


# revision 23
# speedup vs baseline: 4333.8961x; 4333.8961x over previous
"""Trainium2 Bass kernel for CoordsTo_atomNRF.

out[b, i] = (1/(4*MAX_ATOM_NRF)) * sum_{pairs p containing i} atoms_flat[p]*AU2KCALMOLA / r2[b, p]

Per core (batch sharded 8 ways, 1024 examples each):
  - DMA coords b-tiles [128 ex, 192] contiguous; PE transposes (strided
    in-AP per xyz plane) -> atom-major planes C [64, 3, 512] in SBUF.
  - Per chunk of 126 pairs: constant E-matmul (lhsT [64,126] of +-1, f32r)
    forms diffs dx/dy/dz in PSUM; squares+sums split across ACT/DVE/GPSIMD
    (alternating per chunk); custom fast reciprocal on DVE; scatter matmul
    with weights A (atoms_flat * AU2K / 400 folded in) accumulates per-atom
    output in PSUM.
  - Output kept atom-major [64, 1024]; host transposes.

Handles two toolchain quirks post-trace: custom-DVE ISA bytes must be
generated explicitly, and every instruction may carry at most one sync wait
(zero for raw ISA) -- surplus waits are split onto same-engine NoOps.
"""

import numpy as np

import concourse.bass as bass
import concourse.mybir as mybir
from concourse.bass_utils import run_bass_kernel_spmd
from concourse.masks import make_identity
from concourse.tile import TileContext

AU2KCALMOLA = 627.5095 * 0.529177
MAX_ATOM_NRF = 100.0
N = 64
NC2 = 2016
CH = 126          # pairs per chunk
NCHUNK = 16
NCORES = 8
B = 8192
BC = B // NCORES  # 1024 examples per core
BG = 512          # examples per matmul group (PSUM bank width in fp32)
NBG = BC // BG    # 2
NBT = BG // 128   # 4 b-tiles per group

F32 = mybir.dt.float32
F32R = mybir.dt.float32r
USE_F32R = True

# tunables for sim sweeps
OPTS = {
    "wpool_bufs": 2,
    "io_bufs": 8,
    "pl_bufs": 2,
    "path": "alt",  # "alt" | "A" | "B"
    "skip_diff": False,
    "skip_elem": False,
    "skip_scatter": False,
    "skip_stage1": False,
    "repeat": 1,
}


def _build_weights(atoms_flat):
    i_idx, j_idx = np.tril_indices(N, -1)
    cp = (atoms_flat.astype(np.float64) * AU2KCALMOLA / (4.0 * MAX_ATOM_NRF)).astype(
        np.float32
    )
    p = np.arange(NC2)
    c, m = p // CH, p % CH
    ew = np.zeros((2 * N, NC2), np.float32)
    ew[i_idx, p] = 1.0
    ew[j_idx, p] = -1.0
    ew[N + i_idx, p] = 1.0
    ew[N + j_idx, p] = -1.0
    aw = np.zeros((CH, NCHUNK * N), np.float32)
    np.add.at(aw, (m, c * N + i_idx), cp)
    np.add.at(aw, (m, c * N + j_idx), cp)
    return ew, aw


_SQADD = None


def _get_sqadd():
    """Register a custom DVE op: out = Src0*Src0 + Src1 (sha computed at
    runtime so we don't depend on pinned hashes)."""
    global _SQADD
    if _SQADD is not None:
        return _SQADD
    from concourse import dve_ops
    from concourse.dve_spec import Spec, Src0, Src1, _has_src1, lower
    from concourse.dve_table_gen import dve_ver_for
    from concourse.dve_uop import DveOpSpec

    name = "SQADD_KRN"
    existing = [op for op in dve_ops.OPS if op.name == name]
    if existing:
        _SQADD = existing[0]
        return _SQADD

    spec = Spec(
        body=Src0 * Src0 + Src1,
        reference=lambda in0, in1, s0, s1, imm2: (
            in0.astype(np.float32) * in0.astype(np.float32) + in1
        ).astype(np.float32),
    )
    row = max(dve_ops._SUB_OPCODE_FOR_NAME.values()) + 1
    shas = {}
    for ver in ("v3", "v4"):
        try:
            uops = lower(spec, ver=ver)
            shas[ver] = DveOpSpec(
                name=name, opcode=row, uops=uops, rd1_en=_has_src1(spec)
            ).sha(ver)
        except Exception:
            pass
    op = dve_ops.DveOp(name, spec, subdim=False, uops_sha=shas)
    dve_ops.OPS.append(op)
    dve_ops.CUSTOM_DVE_SPECS[name] = spec
    dve_ops._SUB_OPCODE_FOR_NAME[name] = row
    _SQADD = op
    return op


_NC = None


def _get_nc():
    global _NC
    if _NC is None:
        _NC = _build_nc()
        mybir.codegen_inst_isa_subclasses(_NC)  # fill custom-DVE ISA bytes
        _split_waits(_NC)
    return _NC


def _split_waits(nc):
    """This toolchain allows at most one sync wait per instruction (zero on
    raw ISA instructions). Move surplus waits onto same-engine NoOps."""
    n = 0
    for fn in nc.m.functions:
        for bb in fn.blocks:
            insts = list(bb.instructions)
            out = []
            changed = False
            for inst in insts:
                si = inst.sync_info
                maxw = 0 if inst.opcode == "ISA" else 1
                if si is not None and si.on_wait and len(si.on_wait) > maxw:
                    waits = list(si.on_wait)
                    keep = waits[len(waits) - maxw :] if maxw else []
                    move = waits[: len(waits) - maxw] if maxw else waits
                    for w in move:
                        nop = mybir.InstNoOp(
                            name=f"{inst.name}-pw{n}", ins=[], outs=[]
                        )
                        n += 1
                        nop.engine = inst.engine
                        nop.sync_info = mybir.SyncInfo(on_wait=[w], on_update=[])
                        out.append(nop)
                    si.on_wait = keep
                    changed = True
                out.append(inst)
            if changed:
                bb.instructions = out


def _r(ap):
    return ap.bitcast(F32R) if USE_F32R else ap




def _recip_fast(nc, inv, r2):
    """reciprocal_approx_fast with an f32r-typed output tile (the stock
    wrapper asserts fp32 out; the ISA op itself doesn't care)."""
    from concourse.dve_ops import RECIP_APPROX_FAST_CONSTS, RECIPROCAL_APPROX_FAST

    c = RECIP_APPROX_FAST_CONSTS
    nc.vector._custom_dve(
        RECIPROCAL_APPROX_FAST,
        out=inv[:],
        in0=r2[:],
        s0=c["s0"],
        s1=c["s1"],
        imm2=c["imm2"],
    )

def _build_nc():
    sqadd = _get_sqadd()
    nc = bass.Bass()
    coords_d = nc.declare_dram_parameter("coords", [BC, N * 3], F32, isOutput=False)
    ew_d = nc.declare_dram_parameter("ew", [2 * N, NC2], F32R, isOutput=False)
    aw_d = nc.declare_dram_parameter("aw", [CH, NCHUNK * N], F32R, isOutput=False)
    out_d = nc.declare_dram_parameter("out", [N, BC], F32, isOutput=True)

    Sq = mybir.ActivationFunctionType.Square

    with TileContext(nc) as tc:
        with (
            tc.tile_pool(name="const", bufs=1) as cpool,
            tc.tile_pool(name="io", bufs=OPTS["io_bufs"]) as iopool,
            tc.tile_pool(name="planes", bufs=OPTS["pl_bufs"]) as plpool,
            tc.tile_pool(name="work", bufs=OPTS["wpool_bufs"]) as wpool,
            tc.tile_pool(name="invp", bufs=NCHUNK + 2) as ipool,
            tc.tile_pool(name="r2p", bufs=NCHUNK + 2) as r2pool,
            tc.tile_pool(name="pst", bufs=1, space="PSUM") as pst,
            tc.tile_pool(name="psdx", bufs=2, space="PSUM") as psdx,
            tc.tile_pool(name="psdy", bufs=2, space="PSUM") as psdy,
            tc.tile_pool(name="psdz", bufs=2, space="PSUM") as psdz,
            tc.tile_pool(name="pso", bufs=1, space="PSUM") as pso,
        ):
            ident = cpool.tile([128, 128], F32)
            make_identity(nc, ident[:])
            # prefetch all coords b-tiles before the big weight DMAs so the
            # transpose pipeline starts immediately
            raws = []
            for gt in range(NBG * NBT):
                raw = iopool.tile([128, N * 3], F32)
                nc.sync.dma_start(
                    out=raw[:], in_=coords_d[gt * 128 : (gt + 1) * 128, :]
                )
                raws.append(raw)
            ew_sb = cpool.tile([2 * N, NC2], F32R)
            nc.sync.dma_start(out=ew_sb[:], in_=ew_d[:])
            aw_sb = cpool.tile([CH, NCHUNK * N], F32R)
            nc.sync.dma_start(out=aw_sb[:], in_=aw_d[:])

            for rep in range(OPTS["repeat"]):
              if rep > 0:
                raws = []
                for gt in range(NBG * NBT):
                    raw = iopool.tile([128, N * 3], F32)
                    nc.sync.dma_start(
                        out=raw[:], in_=coords_d[gt * 128 : (gt + 1) * 128, :]
                    )
                    raws.append(raw)
              for g in range(NBG):
                # stage 1: load + transpose to atom-major planes, splitting
                # fp32 coords into f32r hi (ACT copy rounds) + exact residual
                # lo (DVE sub) so two 1-cyc/row f32r matmuls reproduce full
                # fp32 diffs.
                Cb = plpool.tile([2 * N, 3, BG], F32R)
                for t in range(NBT):
                    raw = raws[g * NBT + t]
                    rawv = raw[:].rearrange("p (i k) -> p k i", k=3)  # [128,3,64]
                    tp = pst.tile([N, 3 * 128], F32)
                    tpv = tp[:].rearrange("p (k c) -> p k c", k=3)
                    for k in range(3 if not OPTS["skip_stage1"] else 0):
                        nc.tensor.transpose(
                            out=tpv[:, k, :],
                            in_=rawv[:, k, :],
                            identity=ident[:],
                        )
                    if not OPTS["skip_stage1"]:
                        hi = Cb[0:N, :, t * 128 : (t + 1) * 128]
                        nc.scalar.copy(out=hi, in_=tpv[:])
                        nc.vector.tensor_tensor(
                            out=Cb[N : 2 * N, :, t * 128 : (t + 1) * 128],
                            in0=tpv[:],
                            in1=hi.bitcast(F32),
                            op=mybir.AluOpType.subtract,
                        )

                # stage 2: chunk loop (scatters deferred so the in-order
                # PE queue never head-blocks on the elementwise chain)
                outp = pso.tile([N, BG], F32)
                invs = []
                r2s = []
                for c in range(NCHUNK):
                    dxt = psdx.tile([CH, BG], F32)
                    dyt = psdy.tile([CH, BG], F32)
                    dzt = psdz.tile([CH, BG], F32)
                    for k, dt_ in enumerate((dxt, dyt, dzt)):
                        if OPTS["skip_diff"]:
                            break
                        nc.tensor.matmul(
                            out=dt_[:],
                            lhsT=ew_sb[:, c * CH : (c + 1) * CH],
                            rhs=Cb[:, k, :],
                            start=True,
                            stop=True,
                        )
                    dx = dxt[:]
                    dy = dyt[:]
                    dz = dzt[:]
                    if OPTS["skip_elem"]:
                        continue
                    inv = ipool.tile([CH, BG], F32R)
                    # balanced split: ACT squares x/z, DVE fuses y-square+add
                    # and the reciprocal, GPSIMD does the final combine
                    sqx = wpool.tile([CH, BG], F32)
                    sqz = wpool.tile([CH, BG], F32)
                    u = wpool.tile([CH, BG], F32)
                    r2 = r2pool.tile([CH, BG], F32)
                    nc.scalar.activation(sqx[:], dx, Sq)
                    nc.scalar.activation(sqz[:], dz, Sq)
                    nc.vector._custom_dve(sqadd, out=u[:], in0=dy, in1=sqx[:])
                    nc.gpsimd.tensor_add(out=r2[:], in0=u[:], in1=sqz[:])
                    r2s.append((r2, inv))
                    invs.append(inv)
                for r2, inv in r2s:
                    _recip_fast(nc, inv, r2)
                for c in range(NCHUNK if not OPTS["skip_scatter"] else 0):
                    nc.tensor.matmul(
                        out=outp[:],
                        lhsT=aw_sb[:, c * N : (c + 1) * N],
                        rhs=invs[c][:],
                        start=(c == 0),
                        stop=(c == NCHUNK - 1),
                    )

                if not OPTS["skip_scatter"]:
                    osb = iopool.tile([N, BG], F32)
                    nc.scalar.copy(out=osb[:], in_=outp[:])
                    nc.sync.dma_start(
                        out=out_d[:, g * BG : (g + 1) * BG], in_=osb[:]
                    )

    return nc


def run(coords, atoms_flat, trace=False):
    coords = np.ascontiguousarray(np.asarray(coords, dtype=np.float32))
    atoms_flat = np.asarray(atoms_flat, dtype=np.float32)
    assert coords.shape == (B, N, 3), coords.shape
    ew, aw = _build_weights(atoms_flat)
    nc = _get_nc()
    in_maps = [
        {
            "coords": coords[c * BC : (c + 1) * BC].reshape(BC, N * 3),
            "ew": ew,
            "aw": aw,
        }
        for c in range(NCORES)
    ]
    res = run_bass_kernel_spmd(nc, in_maps, list(range(NCORES)), trace=trace)
    out = np.empty((B, N), np.float32)
    for c in range(NCORES):
        out[c * BC : (c + 1) * BC] = res.results[c]["out"].T
    return out, res


def kernel(coords, atoms_flat):
    out, _ = run(coords, atoms_flat, trace=False)
    return out
